# revision 23
# baseline (speedup 1.0000x reference)
"""MoE layer (8 experts, top-2) on 8 Trainium2 NeuronCores.

Strategy (expert parallelism, per the sharding hint):
  Launch 1 (router): tokens data-parallel across the 8 cores, each core
    computes its slice of router logits in true fp32 on the PE.
  Host dispatch:     softmax/top-2/combine-weights replicated from the
    reference in fp32 on the host (0.3 MFLOP of control logic), tokens
    gathered per expert (capacity padded).
  Launch 2 (experts): core e holds expert e's weights; computes
    y = (relu(x @ W1^T)^2 @ W2^T) * w for its gathered tokens.
    Matmuls run in fp16 (fp32 PSUM accumulation); weight loads overlap
    with streaming because 2-byte stationaries use the background
    weight buffer.
  Host combine:      scatter-add of the two expert contributions per
    token, ascending expert order (same fp32 summation order as the
    reference loop).

All matmul FLOPs run on device. Host does data movement + top-2 dispatch.
"""

import numpy as np

N_EXPERTS = 8
TOP_K = 2
N_EMBD = 1024
EXPERT_DIM = 2048
N_TOKENS = 8192          # 4 * 2048
N_CORES = 8
TOK_PER_CORE = N_TOKENS // N_CORES  # 1024 (router shard)
CAP = 2176               # per-expert token capacity (17*128; max observed
                         # count is 2175 for the fixed seed). If routing ever
                         # assigns more than CAP tokens to one expert, the
                         # host runs a second expert pass for the overflow
                         # (correct for any input, never triggered here).
TCH = 256                # expert-kernel token chunk (multiple of 128, max 512
                         # = fp32 PSUM bank limit on the matmul free dim;
                         # 256 measured fastest)

_CACHE = {}


def _build_router_module(repeat=1):
    """Computes logitsT [E, T] = router_w @ x^T in true fp32.

    lhsT = rwT [d, E] is the stationary operand (only 8 weight loads
    total); x^T streams as wide [128, 512] moving tiles.
    """
    import concourse.bacc as bacc
    import concourse.mybir as mybir
    import concourse.tile as tile

    f32 = mybir.dt.float32
    D = N_EMBD
    E = N_EXPERTS
    T = TOK_PER_CORE
    KC = D // 128   # 8 contraction chunks
    TT = 512        # moving-tile token width
    NT = T // TT    # 2 token tiles

    nc = bacc.Bacc("TRN2", target_bir_lowering=False, debug=False,
                   num_devices=N_CORES)
    xT = nc.dram_tensor("xT", [D, T], f32, kind="ExternalInput").ap()
    rwT = nc.dram_tensor("rwT", [D, E], f32, kind="ExternalInput").ap()
    logitsT = nc.dram_tensor("logitsT", [E, T], f32, kind="ExternalOutput").ap()

    with tile.TileContext(nc) as tc:
        with (
            tc.tile_pool(name="wpool", bufs=1) as wpool,
            tc.tile_pool(name="xpool", bufs=2) as xpool,
            tc.tile_pool(name="opool", bufs=2) as opool,
            tc.tile_pool(name="pspool", bufs=2, space="PSUM") as pspool,
        ):
            # router weights, all 8 d-chunks side by side: block k = rwT[k*128:(k+1)*128, :]
            rw_tile = wpool.tile([128, KC * E], f32, tag="rw")
            for k in range(KC):
                nc.sync.dma_start(rw_tile[:, k * E:(k + 1) * E],
                                  rwT[k * 128:(k + 1) * 128, :])

            def body(_=None):
                # whole x^T core slice as one tile: block k = [128(d), T(t)],
                # loaded with 8 full-row DMAs (4KB per partition each)
                x_tile = xpool.tile([128, KC * T], f32, tag="x", name="x")
                for k in range(KC):
                    nc.sync.dma_start(
                        x_tile[:, k * T:(k + 1) * T],
                        xT[k * 128:(k + 1) * 128, :])
                for tt in range(NT):
                    pl = pspool.tile([E, TT], f32, tag="pl", name=f"pl_{tt}")
                    for k in range(KC):
                        nc.tensor.matmul(
                            pl[:],
                            rw_tile[:, k * E:(k + 1) * E],              # lhsT [d, e]
                            x_tile[:, k * T + tt * TT:
                                   k * T + (tt + 1) * TT],              # rhs [d, t]
                            start=(k == 0), stop=(k == KC - 1))
                    ot = opool.tile([E, TT], f32, tag="o", name=f"o_{tt}")
                    nc.scalar.copy(ot[:], pl[:])
                    nc.sync.dma_start(logitsT[:, tt * TT:(tt + 1) * TT], ot[:])

            if repeat == 1:
                body()
            else:
                with tc.For_i(0, repeat, 1) as _i:
                    body(_i)
    nc.compile()
    return nc


def _build_expert_module(repeat=1, mm1="f32r", mm2="f32r"):
    import concourse.bacc as bacc
    import concourse.mybir as mybir
    import concourse.tile as tile

    f32 = mybir.dt.float32
    dt_mm1 = mybir.dt.float32r if mm1 == "f32r" else mybir.dt.float16
    dt_mm2 = mybir.dt.float32r if mm2 == "f32r" else mybir.dt.float16
    D = N_EMBD
    F = EXPERT_DIM
    KD = D // 128     # 8 d-chunks
    KF = F // 128     # 16 f-chunks

    nc = bacc.Bacc("TRN2", target_bir_lowering=False, debug=False,
                   num_devices=N_CORES)
    xT = nc.dram_tensor("xT", [D, CAP], dt_mm1, kind="ExternalInput").ap()
    w1T = nc.dram_tensor("w1T", [D, F], dt_mm1, kind="ExternalInput").ap()
    w2T = nc.dram_tensor("w2T", [F, D], dt_mm2, kind="ExternalInput").ap()
    wv = nc.dram_tensor("wv", [CAP, 1], f32, kind="ExternalInput").ap()
    y = nc.dram_tensor("y", [CAP, D], f32, kind="ExternalOutput").ap()

    with tile.TileContext(nc) as tc:
        with (
            tc.tile_pool(name="wpool", bufs=1) as wpool,
            tc.tile_pool(name="xpool", bufs=3) as xpool,
            tc.tile_pool(name="hpool", bufs=2) as hpool,
            tc.tile_pool(name="rpool", bufs=3) as rpool,
            tc.tile_pool(name="ypool", bufs=3) as ypool,
            tc.tile_pool(name="ph_pool", bufs=4, space="PSUM") as ph_pool,
            tc.tile_pool(name="py_pool", bufs=3, space="PSUM") as py_pool,
        ):
            # chunk list: (token base, chunk width); widths are multiples
            # of 128 and at most 512 (fp32 PSUM bank limit on N)
            chunks = []
            base = 0
            while base < CAP:
                w = min(TCH, CAP - base)
                chunks.append((base, w))
                base += w

            def load_x_chunk(c, cb, cw):
                # x^T chunk: block k = xT[k*128:(k+1)*128, chunk tokens]
                x_tile = xpool.tile([128, KD * cw], dt_mm1, tag="x",
                                    name=f"x_{c}")
                for k in range(KD):
                    nc.sync.dma_start(
                        x_tile[:, k * cw:(k + 1) * cw],
                        xT[k * 128:(k + 1) * 128, cb:cb + cw])
                return x_tile

            # --- resident weights ---
            # DMA issue order shapes the queue order: first the W1 column
            # slices chunk 0's mm1 needs, then chunk 0's x, then the rest
            # (W2 is first consumed ~25us in, after chunk 0's mm1).
            # W1^T d-chunk k: [128(d), F]
            w1_tiles = [
                wpool.tile([128, F], dt_mm1, tag=f"w1_{k}", name=f"w1_{k}")
                for k in range(KD)
            ]
            x0_tile = load_x_chunk(0, chunks[0][0], chunks[0][1])
            for q in range(4):
                for k in range(KD):
                    nc.sync.dma_start(
                        w1_tiles[k][:, q * (F // 4):(q + 1) * (F // 4)],
                        w1T[k * 128:(k + 1) * 128, q * (F // 4):(q + 1) * (F // 4)])
                if q == 0:
                    x1_tile = load_x_chunk(1, chunks[1][0], chunks[1][1])
            # W2^T f-chunk k: [128(f), D]
            w2_tiles = []
            for k in range(KF):
                t = wpool.tile([128, D], dt_mm2, tag=f"w2_{k}", name=f"w2_{k}")
                nc.sync.dma_start(t[:], w2T[k * 128:(k + 1) * 128, :])
                w2_tiles.append(t)
            # combine weights: column j = tokens [j*128, (j+1)*128)
            wv_tile = wpool.tile([128, CAP // 128], f32, tag="wv", name="wv")
            for j in range(CAP // 128):
                nc.sync.dma_start(wv_tile[:, j:j + 1],
                                  wv[j * 128:(j + 1) * 128, :])

            def body(_=None, preloaded=()):
                for c, (cb, cw) in enumerate(chunks):
                    if c < len(preloaded):
                        x_tile = preloaded[c]
                    else:
                        x_tile = load_x_chunk(c, cb, cw)
                    # h^T chunk: block f = [128(f), cw]
                    h_tile = hpool.tile([128, KF * cw], dt_mm2, tag="h",
                                        name=f"h_{c}")
                    for f in range(KF):
                        ph = ph_pool.tile([128, cw], f32, tag="ph",
                                          name=f"ph_{c}_{f}")
                        for k in range(KD):
                            nc.tensor.matmul(
                                ph[:],
                                w1_tiles[k][:, f * 128:(f + 1) * 128],
                                x_tile[:, k * cw:(k + 1) * cw],
                                start=(k == 0), stop=(k == KD - 1))
                        hr = rpool.tile([128, cw], f32, tag="hr",
                                        name=f"hr_{c}_{f}")
                        nc.vector.tensor_scalar_max(hr[:], ph[:], 0.0)
                        nc.scalar.square(h_tile[:, f * cw:(f + 1) * cw], hr[:])
                    for s in range(cw // 128):
                        yt = ypool.tile([128, D], f32, tag="y",
                                        name=f"y_{c}_{s}")
                        for dn in range(D // 512):
                            py = py_pool.tile([128, 512], f32, tag="py",
                                              name=f"py_{c}_{s}_{dn}")
                            for f in range(KF):
                                nc.tensor.matmul(
                                    py[:],
                                    h_tile[:, f * cw + s * 128:
                                           f * cw + (s + 1) * 128],
                                    w2_tiles[f][:, dn * 512:(dn + 1) * 512],
                                    start=(f == 0), stop=(f == KF - 1))
                            nc.scalar.mul(yt[:, dn * 512:(dn + 1) * 512], py[:],
                                          wv_tile[:, (cb + s * 128) // 128:
                                                  (cb + s * 128) // 128 + 1])
                        nc.sync.dma_start(
                            y[cb + s * 128:cb + (s + 1) * 128, :],
                            yt[:])

            if repeat == 1:
                body(preloaded=(x0_tile, x1_tile))
            else:
                with tc.For_i(0, repeat, 1) as _i:
                    body(_i)
    nc.compile()
    return nc


# dtype plan for the two expert matmuls: "f32r" (FP22) or "f16".
# fp16 RNE quantization measures only ~2x the error of f32r truncation
# (rel 4.3e-4 vs 2.1e-4) and allows standalone, overlapped weight loads
# (4-byte matmuls pay a serial ~107ns self-load per matmul).
EXPERT_MM1 = "f16"
EXPERT_MM2 = "f16"


def _get_module(name):
    if name not in _CACHE:
        if name == "router":
            _CACHE[name] = _build_router_module()
        elif name == "expert":
            _CACHE[name] = _build_expert_module(mm1=EXPERT_MM1, mm2=EXPERT_MM2)
        else:
            raise KeyError(name)
    return _CACHE[name]


def _routing_from_logits(logits):
    """Replicates reference softmax/top-2/normalize in fp32 numpy.

    jax.lax.top_k tie-break (lower index first) == stable argsort on -p.
    """
    logits = logits.astype(np.float32, copy=False)
    m = logits.max(axis=1, keepdims=True)
    p = np.exp(logits - m)
    p = (p / p.sum(axis=1, keepdims=True)).astype(np.float32)
    order = np.argsort(-p, axis=1, kind="stable")
    t1 = order[:, 0].astype(np.int32)
    t2 = order[:, 1].astype(np.int32)
    ar = np.arange(logits.shape[0])
    tv1 = p[ar, t1]
    tv2 = p[ar, t2]
    s = (tv1 + tv2).astype(np.float32)
    w1 = (tv1 / s).astype(np.float32)
    w2 = (tv2 / s).astype(np.float32)
    return t1, t2, w1, w2


def kernel(x, router_w, fc1_w, fc2_w):
    from concourse.bass_utils import run_bass_kernel_spmd

    x = np.ascontiguousarray(np.asarray(x, dtype=np.float32))
    router_w = np.ascontiguousarray(np.asarray(router_w, dtype=np.float32))
    fc1_w = np.asarray(fc1_w, dtype=np.float32)
    fc2_w = np.asarray(fc2_w, dtype=np.float32)

    B, T, D = x.shape
    xf = x.reshape(B * T, D)
    xT = np.ascontiguousarray(xf.T)               # [D, N]
    rwT = np.ascontiguousarray(router_w.T)        # [D, E]

    # --- launch 1: router logits on device (true fp32) ---
    nc_r = _get_module("router")
    in_maps = [
        {"xT": np.ascontiguousarray(xT[:, c * TOK_PER_CORE:(c + 1) * TOK_PER_CORE]),
         "rwT": rwT}
        for c in range(N_CORES)
    ]
    res = run_bass_kernel_spmd(nc_r, in_maps, core_ids=list(range(N_CORES)))
    logits = np.concatenate(
        [np.ascontiguousarray(r["logitsT"].T) for r in res.results], axis=0)
    global _LAST_LOGITS
    _LAST_LOGITS = logits

    # --- host dispatch ---
    t1, t2, w1, w2 = _routing_from_logits(logits)
    idx_e = []
    wv_e = []
    for e in range(N_EXPERTS):
        sel = np.where((t1 == e) | (t2 == e))[0]
        idx_e.append(sel)
        wv_e.append(np.where(t1[sel] == e, w1[sel], w2[sel]).astype(np.float32))

    # --- launch 2: expert FFN on device ---
    nc_e = _get_module("expert")
    np1 = np.float32 if EXPERT_MM1 == "f32r" else np.float16
    np2 = np.float32 if EXPERT_MM2 == "f32r" else np.float16
    w1T_np = [np.ascontiguousarray(fc1_w[e].T).astype(np1) for e in range(N_EXPERTS)]
    w2T_np = [np.ascontiguousarray(fc2_w[e].T).astype(np2) for e in range(N_EXPERTS)]
    out = np.zeros((B * T, D), np.float32)
    n_passes = max(1, -(-max(len(s) for s in idx_e) // CAP))
    for p in range(n_passes):  # overflow fallback: extra passes never trigger
        in_maps = []           # for the fixed problem size (max count 2175)
        for e in range(N_EXPERTS):
            sl = idx_e[e][p * CAP:(p + 1) * CAP]
            xg = np.zeros((D, CAP), np1)
            xg[:, :len(sl)] = xT[:, sl].astype(np1)
            wvg = np.zeros((CAP, 1), np.float32)
            wvg[:len(sl), 0] = wv_e[e][p * CAP:(p + 1) * CAP]
            in_maps.append({"xT": xg, "w1T": w1T_np[e], "w2T": w2T_np[e],
                            "wv": wvg})
        res = run_bass_kernel_spmd(nc_e, in_maps, core_ids=list(range(N_CORES)))
        # host combine (ascending expert order == reference accumulation order)
        for e in range(N_EXPERTS):
            sl = idx_e[e][p * CAP:(p + 1) * CAP]
            out[sl] += res.results[e]["y"][:len(sl)]
    return out.reshape(B, T, D)
